# revision 31
# baseline (speedup 1.0000x reference)
"""HMLC loss kernel for 8 Trainium2 NeuronCores (Bass/Tile).

Strategy (queue-sharded data parallelism, fp8 matmul, class-segment stats):
  * All mask/dedup/queue-evolution logic in the reference depends ONLY on the
    integer labels -> computed exactly on host (numpy).
  * The queue (32768 cols) is split into 16 shards (8 cores x 2 vshards).
    Within each shard, columns are ordered by "lifetime class" (the last
    level at which the column is still active, 3..1), assigned round-robin
    over the class-sorted global order so per-class counts differ by at most
    1 across shards ("wobble" of one column at each class boundary).
  * Key fact used for the possum scans: a queue column can be matched at
    level l by SOME anchor iff it dies after level l (class == l); and
    match(i,j) at level l != 0 implies column j is matched at level l.
    Hence pos_l = sum_j (kq_l==ka_i)*sim over ONLY the class-l segment.
  * Device per (vshard, anchor-chunk): PE computes sim = (f/TEMP) @ fq_shard.T
    into PSUM [128, 2048] via fp8e4 DoubleRow matmuls (fp8 noise averages
    out; final loss rel err ~1e-5); then
        E      = exp(sim - CBIAS)            (ScalarE, bf16 out, 1 pass)
        pos_l  = stt (kq_l==ka)*sim, class-l segment   (VectorE, accum)
        s3/s2/s1 = segment sums of E          (VectorE tensor_reduce)
        xA/xB  = E at the two wobble columns  (ScalarE copy)
  * Host combines segments + wobble flags into exact per-shard denominators,
    merges shards in float64, and runs the scalar hmce chain.
"""

import os
import sys
import time
from contextlib import ExitStack

if "/opt/trn_rl_repo" not in sys.path:
    sys.path.insert(0, "/opt/trn_rl_repo")

import numpy as np
import ml_dtypes

import concourse.bass as bass  # noqa: E402
import concourse.bacc as bacc  # noqa: E402
import concourse.tile as tile  # noqa: E402
from concourse import mybir  # noqa: E402
from concourse.bass_utils import run_bass_kernel_spmd  # noqa: E402

TEMP = 0.07
BASE_TEMP = 0.07
NCORES = 8
NVS = 2          # vshards per core
P = 128          # partitions
# |sim| <= (1/TEMP) since features are L2-normalized -> a constant softmax
# shift is numerically safe and removes the per-row reduce_max entirely
CBIAS = 15.0

# populated by kernel() for test harness introspection
LAST_RUN = {}


# ---------------------------------------------------------------- host masks
def _host_masks(labels, labels_queue):
    """Exact replication of the reference's label-only mask evolution."""
    B, L = labels.shape
    Q = labels_queue.shape[0]
    base = int(max(labels.max(), labels_queue.max())) + 1
    pw = base ** np.arange(L - 1, -1, -1)

    anchor_active = np.ones(B, bool)
    queue_active = np.ones(Q, bool)
    order = np.arange(B)

    levels = []
    for l in range(1, L):
        ncols = L - l
        w = (pw * (np.arange(L) < ncols)).astype(np.int64)
        ka = labels.astype(np.int64) @ w
        kq = labels_queue.astype(np.int64) @ w
        maxk = int(max(ka.max(), kq.max())) + 1
        bc = np.bincount(kq[queue_active], minlength=maxk)
        cnt = np.where(anchor_active, bc[ka], 0)
        pres = np.zeros(maxk, bool)
        pres[ka[anchor_active]] = True
        newmatch = queue_active & pres[kq]
        levels.append(dict(
            ka=ka.copy(), kq=kq.copy(),
            queue_active=queue_active.copy(),
            cnt=cnt.copy(),
        ))
        same = (ka[:, None] == ka[None, :]) & anchor_active[:, None] & anchor_active[None, :]
        max_ord = np.max(np.where(same, order[None, :], -1), axis=1)
        kept = anchor_active & (order == max_ord)
        rank = (kept[None, :] & (ka[None, :] < ka[:, None])).sum(1)
        order = np.where(kept, rank, -1)
        anchor_active = kept
        queue_active = queue_active & ~newmatch
    return levels


# ------------------------------------------------------------ device program
def _build_program(D, B, W, n3, n2b):
    NLEV = 3
    f32 = mybir.dt.float32
    bf16 = mybir.dt.bfloat16
    e4 = mybir.dt.float8e4
    DR = mybir.MatmulPerfMode.DoubleRow
    NB = B // P       # anchor chunks
    NK = D // P       # contraction chunks
    NGR = W // 512    # moving groups per vshard

    # possum scan ranges (class-l segment incl the wobble column)
    R3 = (0, min(n3 + 1, W))
    R2 = (n3, min(n2b + 1, W))
    R1 = (n2b, W)
    L3 = R3[1] - R3[0]
    L2 = R2[1] - R2[0]
    L1 = R1[1] - R1[0]

    s2a, s2b = n3 + 1, n2b
    L2s = s2b - s2a

    nc = bacc.Bacc("TRN2", target_bir_lowering=False, debug=False)

    ft_d = nc.dram_tensor("ft", [D, B], e4, kind="ExternalInput").ap()
    fqt_d = nc.dram_tensor("fqt", [D, NVS * W], e4, kind="ExternalInput").ap()
    ka_d = nc.dram_tensor("ka", [NLEV, P, NB], f32, kind="ExternalInput").ap()
    # kq arrays pre-replicated to 128 partitions on host (direct HW-DGE DMA
    # instead of a software-DGE broadcast on the critical path)
    kq3_d = nc.dram_tensor("kq3", [NVS, P, L3], f32, kind="ExternalInput").ap()
    kq2_d = nc.dram_tensor("kq2", [NVS, P, L2], f32, kind="ExternalInput").ap()
    kq1_d = nc.dram_tensor("kq1", [NVS, P, L1], f32, kind="ExternalInput").ap()
    # stats slots: 0:pos3 1:pos2 2:pos1 3:s3 4:xA 5:s2 6:xB 7:den_full
    stats_d = nc.dram_tensor(
        "stats", [NVS, P, 8, NB], f32, kind="ExternalOutput").ap()

    with tile.TileContext(nc) as tc, ExitStack() as ctx:
        const_pool = ctx.enter_context(tc.tile_pool(name="const", bufs=1))
        fqt_pool = ctx.enter_context(tc.tile_pool(name="fqt", bufs=2))
        e_pool = ctx.enter_context(tc.tile_pool(name="ee", bufs=2))
        scr_pool = ctx.enter_context(tc.tile_pool(name="scr", bufs=2))
        s3scr_pool = ctx.enter_context(tc.tile_pool(name="s3s", bufs=2))
        st_pool = ctx.enter_context(tc.tile_pool(name="st", bufs=2))
        psum_pool = ctx.enter_context(tc.tile_pool(name="ps", bufs=2, space="PSUM"))

        ft_sb = const_pool.tile([P, NK, B], e4)
        ft_r = ft_d.rearrange("(k p) b -> p k b", p=P)
        # ft DMAs are interleaved with the first vshard's fqt chunks below
        ka_sb = const_pool.tile([P, NLEV, NB], f32)
        nc.gpsimd.dma_start(out=ka_sb, in_=ka_d.rearrange("l p c -> p l c"))
        cbias_sb = const_pool.tile([P, 1], f32)
        nc.vector.memset(cbias_sb, -CBIAS)

        # all kq key tiles loaded up front (scalar-engine HW-DGE queue)
        kq3b_all, kq2b_all, kq1b_all = [], [], []
        for v in range(NVS):
            t3 = const_pool.tile([P, L3], f32, name=f"kq3_{v}")
            nc.scalar.dma_start(out=t3, in_=kq3_d[v])
            t2 = const_pool.tile([P, L2], f32, name=f"kq2_{v}")
            nc.scalar.dma_start(out=t2, in_=kq2_d[v])
            t1 = const_pool.tile([P, L1], f32, name=f"kq1_{v}")
            nc.scalar.dma_start(out=t1, in_=kq1_d[v])
            kq3b_all.append(t3)
            kq2b_all.append(t2)
            kq1b_all.append(t1)

        for v in range(NVS):
            fqt_sb = fqt_pool.tile([P, NK, W], e4)
            fqt_r = fqt_d[:, v * W:(v + 1) * W].rearrange("(k p) w -> p k w", p=P)
            for k in range(NK):
                nc.sync.dma_start(out=fqt_sb[:, k, :], in_=fqt_r[:, k, :])
                if v == 0:
                    nc.sync.dma_start(out=ft_sb[:, k, :], in_=ft_r[:, k, :])

            kq3b = kq3b_all[v]
            kq2b = kq2b_all[v]
            kq1b = kq1b_all[v]

            stats_v = st_pool.tile([P, 8, NB], f32, tag="stats")

            for c in range(NB):
                ps = psum_pool.tile([P, W], f32)
                for k in range(0, NK, 2):
                    for g in range(NGR):
                        gs = slice(g * 512, (g + 1) * 512)
                        nc.tensor.matmul(
                            ps[:, gs],
                            ft_sb[:, k:k + 2, c * P:(c + 1) * P],
                            fqt_sb[:, k:k + 2, gs],
                            start=(k == 0), stop=(k == NK - 2),
                            perf_mode=DR)

                # E = exp(sim - CBIAS), bf16, one pass over the full width;
                # the accumulator gives den_full (= den1) for free
                E = e_pool.tile([P, W], bf16, tag="E")
                nc.scalar.activation(
                    E, ps, mybir.ActivationFunctionType.Exp,
                    bias=cbias_sb[:, 0:1], scale=1.0,
                    accum_out=stats_v[:, 7, c:c + 1])

                # possum per level over its class segment (VectorE)
                scr = scr_pool.tile([P, L3 + L2 + L1 + L2s], bf16, tag="scr")
                s3scr = s3scr_pool.tile([P, max(n3, 1)], bf16, tag="s3scr")
                nc.vector.scalar_tensor_tensor(
                    out=scr[:, 0:L3], in0=kq3b,
                    scalar=ka_sb[:, 2, c:c + 1], in1=ps[:, R3[0]:R3[1]],
                    op0=mybir.AluOpType.is_equal, op1=mybir.AluOpType.mult,
                    accum_out=stats_v[:, 0, c:c + 1])
                nc.vector.scalar_tensor_tensor(
                    out=scr[:, L3:L3 + L2], in0=kq2b,
                    scalar=ka_sb[:, 1, c:c + 1], in1=ps[:, R2[0]:R2[1]],
                    op0=mybir.AluOpType.is_equal, op1=mybir.AluOpType.mult,
                    accum_out=stats_v[:, 1, c:c + 1])
                nc.vector.scalar_tensor_tensor(
                    out=scr[:, L3 + L2:L3 + L2 + L1], in0=kq1b,
                    scalar=ka_sb[:, 0, c:c + 1], in1=ps[:, R1[0]:R1[1]],
                    op0=mybir.AluOpType.is_equal, op1=mybir.AluOpType.mult,
                    accum_out=stats_v[:, 2, c:c + 1])

                # denominator segments (disjoint; host recombines with wobble)
                # s3 on ScalarE via Copy+accum (small)
                nc.scalar.activation(
                    s3scr[:, 0:n3], E[:, 0:n3],
                    mybir.ActivationFunctionType.Copy,
                    accum_out=stats_v[:, 3, c:c + 1])
                # s2 via tensor_scalar+accum (all-bf16 operands -> 2x-eligible)
                nc.vector.tensor_scalar(
                    out=scr[:, L3 + L2 + L1:L3 + L2 + L1 + L2s],
                    in0=E[:, s2a:s2b], scalar1=1.0, scalar2=0.0,
                    op0=mybir.AluOpType.mult, op1=mybir.AluOpType.add,
                    accum_out=stats_v[:, 5, c:c + 1])
                # wobble columns (ScalarE copy)
                nc.scalar.activation(
                    stats_v[:, 4, c:c + 1], E[:, n3:n3 + 1],
                    mybir.ActivationFunctionType.Copy)
                nc.scalar.activation(
                    stats_v[:, 6, c:c + 1], E[:, n2b:n2b + 1],
                    mybir.ActivationFunctionType.Copy)

            nc.sync.dma_start(out=stats_d[v], in_=stats_v)

    nc.compile()
    return nc


# ----------------------------------------------------------------- host prep
def _prepare(features, labels, features_queue, labels_queue):
    B, D = features.shape
    Q = features_queue.shape[0]
    S = NCORES * NVS
    W = Q // S
    NB = B // P
    NLEV = 3

    levels = _host_masks(labels, labels_queue)
    qa2 = levels[1]["queue_active"]
    qa3 = levels[2]["queue_active"]
    life = 1 + qa2.astype(np.int64) + qa3.astype(np.int64)  # 1..3

    order_cols = np.argsort(-life, kind="stable")
    perm = order_cols.reshape(W, S).T  # [S, W]: shard s -> global cols
    life_s = life[perm]
    c3_s = (life_s == 3).sum(1)
    c23_s = (life_s >= 2).sum(1)
    n3 = int(c3_s.min())
    n2b = int(c23_s.min())
    assert int(c3_s.max()) - n3 <= 1 and int(c23_s.max()) - n2b <= 1
    assert 0 < n3 and n3 + 1 < n2b and n2b + 1 < W
    w3 = (c3_s > n3).astype(np.float64)  # [S] wobble col at n3 is class-3
    w2 = (c23_s > n2b).astype(np.float64)

    R3 = (0, n3 + 1)
    R2 = (n3, n2b + 1)
    R1 = (n2b, W)

    # per-level class-masked queue keys (a column can only match at level l
    # if its class is exactly l)
    kq_cls = {}
    for li, cls in ((0, 1), (1, 2), (2, 3)):
        k = levels[li]["kq"].astype(np.float32)
        kq_cls[li] = np.where(life == cls, k, np.float32(-1.0))[perm]  # [S, W]

    ka_r = np.empty((NLEV, P, NB), np.float32)
    for li in range(NLEV):
        ka_r[li] = levels[li]["ka"].astype(np.float32).reshape(NB, P).T

    e4 = ml_dtypes.float8_e4m3
    ft8 = np.ascontiguousarray((features / TEMP).T).astype(e4)  # [D, B]
    fqT = np.ascontiguousarray(features_queue.T)                # [D, Q]

    in_maps = []
    for cidx in range(NCORES):
        sh = range(cidx * NVS, (cidx + 1) * NVS)
        cols = np.concatenate([perm[s] for s in sh])
        fq8 = np.ascontiguousarray(fqT[:, cols]).astype(e4)
        def _rep(a):  # [NVS, L] -> [NVS, P, L] replicated over partitions
            return np.ascontiguousarray(
                np.broadcast_to(a[:, None, :], (NVS, P, a.shape[1])))
        m = {
            "ft": ft8, "fqt": fq8, "ka": ka_r,
            "kq3": _rep(np.stack([kq_cls[2][s, R3[0]:R3[1]] for s in sh])),
            "kq2": _rep(np.stack([kq_cls[1][s, R2[0]:R2[1]] for s in sh])),
            "kq1": _rep(np.stack([kq_cls[0][s, R1[0]:R1[1]] for s in sh])),
        }
        in_maps.append(m)

    return dict(in_maps=in_maps, levels=levels, perm=perm,
                n3=n3, n2b=n2b, w3=w3, w2=w2,
                B=B, D=D, Q=Q, S=S, W=W, NB=NB, NLEV=NLEV)


# -------------------------------------------------------------------- kernel
def kernel(features, labels, features_queue, labels_queue):
    t0 = time.time()
    features = np.asarray(features, dtype=np.float32)
    features_queue = np.asarray(features_queue, dtype=np.float32)
    labels = np.asarray(labels)
    labels_queue = np.asarray(labels_queue)

    prep = _prepare(features, labels, features_queue, labels_queue)
    in_maps = prep["in_maps"]
    levels = prep["levels"]
    B, D = prep["B"], prep["D"]
    S, W = prep["S"], prep["W"]
    NB, NLEV = prep["NB"], prep["NLEV"]
    n3, n2b = prep["n3"], prep["n2b"]
    w3, w2 = prep["w3"], prep["w2"]
    t_prep = time.time() - t0

    t0 = time.time()
    nc = _build_program(D, B, W, n3, n2b)
    t_build = time.time() - t0

    t0 = time.time()
    br = run_bass_kernel_spmd(nc, in_maps, core_ids=list(range(NCORES)))
    t_run = time.time() - t0

    LAST_RUN.clear()
    LAST_RUN.update(
        exec_time_ns=br.exec_time_ns,
        mean_exec_time_ns=getattr(br, "mean_exec_time_ns", None),
        t_prep=t_prep, t_build=t_build, t_run=t_run,
        profile_json=br.profile_json,
        instructions_and_trace=br.instructions_and_trace,
        n3=n3, n2b=n2b)

    # ---- host merge (float64)
    t0 = time.time()
    pos = np.empty((S, NLEV, B), np.float64)
    den = np.empty((S, NLEV, B), np.float64)
    for cidx in range(NCORES):
        st = br.results[cidx]["stats"].astype(np.float64)  # [NVS, P, 8, NB]
        for v in range(NVS):
            s = cidx * NVS + v
            sv = st[v]  # [P, 8, NB]
            # slots: 0:pos3 1:pos2 2:pos1 3:s3 4:xA 5:s2 6:xB 7:den_full
            for li, slot in ((2, 0), (1, 1), (0, 2)):
                pos[s, li] = sv[:, slot, :].T.reshape(-1)
            s3 = sv[:, 3, :].T.reshape(-1)
            xA = sv[:, 4, :].T.reshape(-1)
            s2 = sv[:, 5, :].T.reshape(-1)
            xB = sv[:, 6, :].T.reshape(-1)
            den_full = sv[:, 7, :].T.reshape(-1)
            den[s, 2] = s3 + w3[s] * xA
            den[s, 1] = s3 + xA + s2 + w2[s] * xB
            den[s, 0] = den_full

    cum = 0.0
    max_lower = -np.inf
    for li in range(NLEV):
        l = li + 1
        cnt = levels[li]["cnt"].astype(np.float64)
        dtot = den[:, li, :].sum(axis=0)
        ptot = pos[:, li, :].sum(axis=0)
        with np.errstate(divide="ignore", invalid="ignore"):
            mean = (ptot - cnt * (CBIAS + np.log(dtot))) / (cnt + 1e-12)
        mean = np.where(cnt > 0, mean, 0.0)
        loss_i = -(TEMP / BASE_TEMP) * mean
        num = float((cnt > 0).sum())
        layer_loss = float(loss_i.sum() / (num + 1e-12))
        layer_loss = max(max_lower, layer_loss)
        cum = cum + (2.0 ** (1.0 / l)) * layer_loss
        max_lower = max(max_lower, layer_loss)

    LAST_RUN["t_merge"] = time.time() - t0
    return np.float32(cum)


# revision 34
# speedup vs baseline: 1.1507x; 1.1507x over previous
"""HMLC loss kernel for 8 Trainium2 NeuronCores (Bass/Tile).

Strategy (queue-sharded data parallelism, fp8 matmul, class-segment stats):
  * All mask/dedup/queue-evolution logic in the reference depends ONLY on the
    integer labels -> computed exactly on host (numpy).
  * The queue (32768 cols) is split into 16 shards (8 cores x 2 vshards).
    Within each shard, columns are ordered by "lifetime class" (the last
    level at which the column is still active, 3..1), assigned round-robin
    over the class-sorted global order so per-class counts differ by at most
    1 across shards ("wobble" of one column at each class boundary).
  * Key fact used for the possum scans: a queue column can be matched at
    level l by SOME anchor iff it dies after level l (class == l); and
    match(i,j) at level l != 0 implies column j is matched at level l.
    Hence pos_l = sum_j (kq_l==ka_i)*sim over ONLY the class-l segment.
  * Device per (vshard, anchor-chunk): PE computes sim = (f/TEMP) @ fq_shard.T
    into PSUM [128, 2048] via fp8e4 DoubleRow matmuls (fp8 noise averages
    out; final loss rel err ~1e-5); then
        E      = exp(sim - CBIAS)            (ScalarE, bf16 out, 1 pass)
        pos_l  = stt (kq_l==ka)*sim, class-l segment   (VectorE, accum)
        s3/s2/s1 = segment sums of E          (VectorE tensor_reduce)
        xA/xB  = E at the two wobble columns  (ScalarE copy)
  * Host combines segments + wobble flags into exact per-shard denominators,
    merges shards in float64, and runs the scalar hmce chain.
"""

import os
import sys
import time
from contextlib import ExitStack

if "/opt/trn_rl_repo" not in sys.path:
    sys.path.insert(0, "/opt/trn_rl_repo")

import numpy as np
import ml_dtypes

import concourse.bass as bass  # noqa: E402
import concourse.bacc as bacc  # noqa: E402
import concourse.tile as tile  # noqa: E402
from concourse import mybir  # noqa: E402
from concourse.bass_utils import run_bass_kernel_spmd  # noqa: E402

TEMP = 0.07
BASE_TEMP = 0.07
NCORES = 8
NVS = 2          # vshards per core
P = 128          # partitions
# |sim| <= (1/TEMP) since features are L2-normalized -> a constant softmax
# shift is numerically safe and removes the per-row reduce_max entirely
CBIAS = 15.0

# populated by kernel() for test harness introspection
LAST_RUN = {}


# ---------------------------------------------------------------- host masks
def _host_masks(labels, labels_queue):
    """Exact replication of the reference's label-only mask evolution."""
    B, L = labels.shape
    Q = labels_queue.shape[0]
    base = int(max(labels.max(), labels_queue.max())) + 1
    pw = base ** np.arange(L - 1, -1, -1)

    anchor_active = np.ones(B, bool)
    queue_active = np.ones(Q, bool)
    order = np.arange(B)

    levels = []
    for l in range(1, L):
        ncols = L - l
        w = (pw * (np.arange(L) < ncols)).astype(np.int64)
        ka = labels.astype(np.int64) @ w
        kq = labels_queue.astype(np.int64) @ w
        maxk = int(max(ka.max(), kq.max())) + 1
        bc = np.bincount(kq[queue_active], minlength=maxk)
        cnt = np.where(anchor_active, bc[ka], 0)
        pres = np.zeros(maxk, bool)
        pres[ka[anchor_active]] = True
        newmatch = queue_active & pres[kq]
        levels.append(dict(
            ka=ka.copy(), kq=kq.copy(),
            queue_active=queue_active.copy(),
            cnt=cnt.copy(),
        ))
        same = (ka[:, None] == ka[None, :]) & anchor_active[:, None] & anchor_active[None, :]
        max_ord = np.max(np.where(same, order[None, :], -1), axis=1)
        kept = anchor_active & (order == max_ord)
        rank = (kept[None, :] & (ka[None, :] < ka[:, None])).sum(1)
        order = np.where(kept, rank, -1)
        anchor_active = kept
        queue_active = queue_active & ~newmatch
    return levels


# ------------------------------------------------------------ device program
def _build_program(D, B, W, n3, n2b):
    NLEV = 3
    f32 = mybir.dt.float32
    bf16 = mybir.dt.bfloat16
    e4 = mybir.dt.float8e4
    DR = mybir.MatmulPerfMode.DoubleRow
    NB = B // P       # anchor chunks
    NK = D // P       # contraction chunks
    NGR = W // 512    # moving groups per vshard

    # possum scan ranges (class-l segment incl the wobble column)
    R3 = (0, min(n3 + 1, W))
    R2 = (n3, min(n2b + 1, W))
    R1 = (n2b, W)
    L3 = R3[1] - R3[0]
    L2 = R2[1] - R2[0]
    L1 = R1[1] - R1[0]

    s2a, s2b = n3 + 1, n2b
    L2s = s2b - s2a

    nc = bacc.Bacc("TRN2", target_bir_lowering=False, debug=False)

    ft_d = nc.dram_tensor("ft", [D, B], e4, kind="ExternalInput").ap()
    fqt_d = nc.dram_tensor("fqt", [D, NVS * W], e4, kind="ExternalInput").ap()
    ka_d = nc.dram_tensor("ka", [NLEV, P, NB], f32, kind="ExternalInput").ap()
    # kq arrays pre-replicated to 128 partitions on host (direct HW-DGE DMA
    # instead of a software-DGE broadcast on the critical path)
    kq3_d = nc.dram_tensor("kq3", [NVS, P, L3], f32, kind="ExternalInput").ap()
    kq2_d = nc.dram_tensor("kq2", [NVS, P, L2], f32, kind="ExternalInput").ap()
    kq1_d = nc.dram_tensor("kq1", [NVS, P, L1], f32, kind="ExternalInput").ap()
    # stats slots: 0:pos3 1:pos2 2:pos1 3:s3 4:xA 5:s2 6:xB 7:den_full
    stats_d = nc.dram_tensor(
        "stats", [NVS, P, 8, NB], f32, kind="ExternalOutput").ap()

    with tile.TileContext(nc) as tc, ExitStack() as ctx:
        const_pool = ctx.enter_context(tc.tile_pool(name="const", bufs=1))
        fqt_pool = ctx.enter_context(tc.tile_pool(name="fqt", bufs=2))
        e_pool = ctx.enter_context(tc.tile_pool(name="ee", bufs=2))
        scr_pool = ctx.enter_context(tc.tile_pool(name="scr", bufs=2))
        s3scr_pool = ctx.enter_context(tc.tile_pool(name="s3s", bufs=2))
        st_pool = ctx.enter_context(tc.tile_pool(name="st", bufs=2))
        psum_pool = ctx.enter_context(tc.tile_pool(name="ps", bufs=2, space="PSUM"))

        ft_sb = const_pool.tile([P, NK, B], e4)
        ft_r = ft_d.rearrange("(k p) b -> p k b", p=P)
        # ft DMAs are interleaved with the first vshard's fqt chunks below
        ka_sb = const_pool.tile([P, NLEV, NB], f32)
        nc.gpsimd.dma_start(out=ka_sb, in_=ka_d.rearrange("l p c -> p l c"))
        cbias_sb = const_pool.tile([P, 1], f32)
        nc.vector.memset(cbias_sb, -CBIAS)

        # all kq key tiles loaded up front (direct copies, software DGE queue)
        kq3b_all, kq2b_all, kq1b_all = [], [], []
        for v in range(NVS):
            t3 = const_pool.tile([P, L3], f32, name=f"kq3_{v}")
            nc.gpsimd.dma_start(out=t3, in_=kq3_d[v])
            t2 = const_pool.tile([P, L2], f32, name=f"kq2_{v}")
            nc.gpsimd.dma_start(out=t2, in_=kq2_d[v])
            t1 = const_pool.tile([P, L1], f32, name=f"kq1_{v}")
            nc.gpsimd.dma_start(out=t1, in_=kq1_d[v])
            kq3b_all.append(t3)
            kq2b_all.append(t2)
            kq1b_all.append(t1)

        for v in range(NVS):
            fqt_sb = fqt_pool.tile([P, NK, W], e4)
            fqt_r = fqt_d[:, v * W:(v + 1) * W].rearrange("(k p) w -> p k w", p=P)
            for k in range(NK):
                nc.sync.dma_start(out=fqt_sb[:, k, :], in_=fqt_r[:, k, :])
                if v == 0:
                    nc.sync.dma_start(out=ft_sb[:, k, :], in_=ft_r[:, k, :])

            kq3b = kq3b_all[v]
            kq2b = kq2b_all[v]
            kq1b = kq1b_all[v]

            stats_v = st_pool.tile([P, 8, NB], f32, tag="stats")

            for c in range(NB):
                ps = psum_pool.tile([P, W], f32)
                for k in range(0, NK, 2):
                    for g in range(NGR):
                        gs = slice(g * 512, (g + 1) * 512)
                        nc.tensor.matmul(
                            ps[:, gs],
                            ft_sb[:, k:k + 2, c * P:(c + 1) * P],
                            fqt_sb[:, k:k + 2, gs],
                            start=(k == 0), stop=(k == NK - 2),
                            perf_mode=DR)

                # E = exp(sim - CBIAS), bf16, one pass over the full width;
                # the accumulator gives den_full (= den1) for free
                E = e_pool.tile([P, W], bf16, tag="E")
                nc.scalar.activation(
                    E, ps, mybir.ActivationFunctionType.Exp,
                    bias=cbias_sb[:, 0:1], scale=1.0,
                    accum_out=stats_v[:, 7, c:c + 1])

                # possum per level over its class segment (VectorE)
                scr = scr_pool.tile([P, L3 + L2 + L1], bf16, tag="scr")
                s3scr = s3scr_pool.tile([P, max(n3, 1)], bf16, tag="s3scr")
                nc.vector.scalar_tensor_tensor(
                    out=scr[:, 0:L3], in0=kq3b,
                    scalar=ka_sb[:, 2, c:c + 1], in1=ps[:, R3[0]:R3[1]],
                    op0=mybir.AluOpType.is_equal, op1=mybir.AluOpType.mult,
                    accum_out=stats_v[:, 0, c:c + 1])
                nc.vector.scalar_tensor_tensor(
                    out=scr[:, L3:L3 + L2], in0=kq2b,
                    scalar=ka_sb[:, 1, c:c + 1], in1=ps[:, R2[0]:R2[1]],
                    op0=mybir.AluOpType.is_equal, op1=mybir.AluOpType.mult,
                    accum_out=stats_v[:, 1, c:c + 1])
                nc.vector.scalar_tensor_tensor(
                    out=scr[:, L3 + L2:L3 + L2 + L1], in0=kq1b,
                    scalar=ka_sb[:, 0, c:c + 1], in1=ps[:, R1[0]:R1[1]],
                    op0=mybir.AluOpType.is_equal, op1=mybir.AluOpType.mult,
                    accum_out=stats_v[:, 2, c:c + 1])

                # denominator segments (disjoint; host recombines with wobble)
                # s3 on ScalarE via Copy+accum (small)
                nc.scalar.activation(
                    s3scr[:, 0:n3], E[:, 0:n3],
                    mybir.ActivationFunctionType.Copy,
                    accum_out=stats_v[:, 3, c:c + 1])
                # s2 segment sum (VectorE reduce)
                nc.vector.tensor_reduce(
                    out=stats_v[:, 5, c:c + 1], in_=E[:, s2a:s2b],
                    axis=mybir.AxisListType.X, op=mybir.AluOpType.add)
                # wobble columns (ScalarE copy)
                nc.scalar.activation(
                    stats_v[:, 4, c:c + 1], E[:, n3:n3 + 1],
                    mybir.ActivationFunctionType.Copy)
                nc.scalar.activation(
                    stats_v[:, 6, c:c + 1], E[:, n2b:n2b + 1],
                    mybir.ActivationFunctionType.Copy)

            nc.sync.dma_start(out=stats_d[v], in_=stats_v)

    nc.compile()
    return nc


# ----------------------------------------------------------------- host prep
def _prepare(features, labels, features_queue, labels_queue):
    B, D = features.shape
    Q = features_queue.shape[0]
    S = NCORES * NVS
    W = Q // S
    NB = B // P
    NLEV = 3

    levels = _host_masks(labels, labels_queue)
    qa2 = levels[1]["queue_active"]
    qa3 = levels[2]["queue_active"]
    life = 1 + qa2.astype(np.int64) + qa3.astype(np.int64)  # 1..3

    order_cols = np.argsort(-life, kind="stable")
    perm = order_cols.reshape(W, S).T  # [S, W]: shard s -> global cols
    life_s = life[perm]
    c3_s = (life_s == 3).sum(1)
    c23_s = (life_s >= 2).sum(1)
    n3 = int(c3_s.min())
    n2b = int(c23_s.min())
    assert int(c3_s.max()) - n3 <= 1 and int(c23_s.max()) - n2b <= 1
    assert 0 < n3 and n3 + 1 < n2b and n2b + 1 < W
    w3 = (c3_s > n3).astype(np.float64)  # [S] wobble col at n3 is class-3
    w2 = (c23_s > n2b).astype(np.float64)

    R3 = (0, n3 + 1)
    R2 = (n3, n2b + 1)
    R1 = (n2b, W)

    # per-level class-masked queue keys (a column can only match at level l
    # if its class is exactly l)
    kq_cls = {}
    for li, cls in ((0, 1), (1, 2), (2, 3)):
        k = levels[li]["kq"].astype(np.float32)
        kq_cls[li] = np.where(life == cls, k, np.float32(-1.0))[perm]  # [S, W]

    ka_r = np.empty((NLEV, P, NB), np.float32)
    for li in range(NLEV):
        ka_r[li] = levels[li]["ka"].astype(np.float32).reshape(NB, P).T

    e4 = ml_dtypes.float8_e4m3
    ft8 = np.ascontiguousarray((features / TEMP).T).astype(e4)  # [D, B]
    fqT = np.ascontiguousarray(features_queue.T)                # [D, Q]

    in_maps = []
    for cidx in range(NCORES):
        sh = range(cidx * NVS, (cidx + 1) * NVS)
        cols = np.concatenate([perm[s] for s in sh])
        fq8 = np.ascontiguousarray(fqT[:, cols]).astype(e4)
        def _rep(a):  # [NVS, L] -> [NVS, P, L] replicated over partitions
            return np.ascontiguousarray(
                np.broadcast_to(a[:, None, :], (NVS, P, a.shape[1])))
        m = {
            "ft": ft8, "fqt": fq8, "ka": ka_r,
            "kq3": _rep(np.stack([kq_cls[2][s, R3[0]:R3[1]] for s in sh])),
            "kq2": _rep(np.stack([kq_cls[1][s, R2[0]:R2[1]] for s in sh])),
            "kq1": _rep(np.stack([kq_cls[0][s, R1[0]:R1[1]] for s in sh])),
        }
        in_maps.append(m)

    return dict(in_maps=in_maps, levels=levels, perm=perm,
                n3=n3, n2b=n2b, w3=w3, w2=w2,
                B=B, D=D, Q=Q, S=S, W=W, NB=NB, NLEV=NLEV)


# -------------------------------------------------------------------- kernel
def kernel(features, labels, features_queue, labels_queue):
    t0 = time.time()
    features = np.asarray(features, dtype=np.float32)
    features_queue = np.asarray(features_queue, dtype=np.float32)
    labels = np.asarray(labels)
    labels_queue = np.asarray(labels_queue)

    prep = _prepare(features, labels, features_queue, labels_queue)
    in_maps = prep["in_maps"]
    levels = prep["levels"]
    B, D = prep["B"], prep["D"]
    S, W = prep["S"], prep["W"]
    NB, NLEV = prep["NB"], prep["NLEV"]
    n3, n2b = prep["n3"], prep["n2b"]
    w3, w2 = prep["w3"], prep["w2"]
    t_prep = time.time() - t0

    t0 = time.time()
    nc = _build_program(D, B, W, n3, n2b)
    t_build = time.time() - t0

    t0 = time.time()
    br = run_bass_kernel_spmd(nc, in_maps, core_ids=list(range(NCORES)))
    t_run = time.time() - t0

    LAST_RUN.clear()
    LAST_RUN.update(
        exec_time_ns=br.exec_time_ns,
        mean_exec_time_ns=getattr(br, "mean_exec_time_ns", None),
        t_prep=t_prep, t_build=t_build, t_run=t_run,
        profile_json=br.profile_json,
        instructions_and_trace=br.instructions_and_trace,
        n3=n3, n2b=n2b)

    # ---- host merge (float64)
    t0 = time.time()
    pos = np.empty((S, NLEV, B), np.float64)
    den = np.empty((S, NLEV, B), np.float64)
    for cidx in range(NCORES):
        st = br.results[cidx]["stats"].astype(np.float64)  # [NVS, P, 8, NB]
        for v in range(NVS):
            s = cidx * NVS + v
            sv = st[v]  # [P, 8, NB]
            # slots: 0:pos3 1:pos2 2:pos1 3:s3 4:xA 5:s2 6:xB 7:den_full
            for li, slot in ((2, 0), (1, 1), (0, 2)):
                pos[s, li] = sv[:, slot, :].T.reshape(-1)
            s3 = sv[:, 3, :].T.reshape(-1)
            xA = sv[:, 4, :].T.reshape(-1)
            s2 = sv[:, 5, :].T.reshape(-1)
            xB = sv[:, 6, :].T.reshape(-1)
            den_full = sv[:, 7, :].T.reshape(-1)
            den[s, 2] = s3 + w3[s] * xA
            den[s, 1] = s3 + xA + s2 + w2[s] * xB
            den[s, 0] = den_full

    cum = 0.0
    max_lower = -np.inf
    for li in range(NLEV):
        l = li + 1
        cnt = levels[li]["cnt"].astype(np.float64)
        dtot = den[:, li, :].sum(axis=0)
        ptot = pos[:, li, :].sum(axis=0)
        with np.errstate(divide="ignore", invalid="ignore"):
            mean = (ptot - cnt * (CBIAS + np.log(dtot))) / (cnt + 1e-12)
        mean = np.where(cnt > 0, mean, 0.0)
        loss_i = -(TEMP / BASE_TEMP) * mean
        num = float((cnt > 0).sum())
        layer_loss = float(loss_i.sum() / (num + 1e-12))
        layer_loss = max(max_lower, layer_loss)
        cum = cum + (2.0 ** (1.0 / l)) * layer_loss
        max_lower = max(max_lower, layer_loss)

    LAST_RUN["t_merge"] = time.time() - t0
    return np.float32(cum)


# revision 48
# speedup vs baseline: 1.3901x; 1.2080x over previous
"""HMLC loss kernel for 8 Trainium2 NeuronCores (Bass/Tile).

Strategy (queue-sharded data parallelism, fp8 matmul, class-segment stats):
  * All mask/dedup/queue-evolution logic in the reference depends ONLY on the
    integer labels -> computed exactly on host (numpy).
  * The queue (32768 cols) is split into 16 shards (8 cores x 2 vshards).
    Within each shard, columns are ordered by "lifetime class" (the last
    level at which the column is still active, 3..1), assigned round-robin
    over the class-sorted global order so per-class counts differ by at most
    1 across shards ("wobble" of one column at each class boundary).
  * Key fact used for the possum scans: a queue column can be matched at
    level l by SOME anchor iff it dies after level l (class == l); and
    match(i,j) at level l != 0 implies column j is matched at level l.
    Hence pos_l = sum_j (kq_l==ka_i)*sim over ONLY the class-l segment.
  * Device per (vshard, anchor-chunk): PE computes sim = (f/TEMP) @ fq_shard.T
    into PSUM [128, 2048] via fp8e4 DoubleRow matmuls (fp8 noise averages
    out; final loss rel err ~1e-5); then
        E      = exp(sim - CBIAS)            (ScalarE, bf16 out, 1 pass)
        pos_l  = stt (kq_l==ka)*sim, class-l segment   (VectorE, accum)
        s3/s2/s1 = segment sums of E          (VectorE tensor_reduce)
        xA/xB  = E at the two wobble columns  (ScalarE copy)
  * Host combines segments + wobble flags into exact per-shard denominators,
    merges shards in float64, and runs the scalar hmce chain.
"""

import os
import sys
import time
from contextlib import ExitStack

if "/opt/trn_rl_repo" not in sys.path:
    sys.path.insert(0, "/opt/trn_rl_repo")

import numpy as np
import ml_dtypes

import concourse.bass as bass  # noqa: E402
import concourse.bacc as bacc  # noqa: E402
import concourse.tile as tile  # noqa: E402
from concourse import mybir  # noqa: E402
from concourse.bass_utils import run_bass_kernel_spmd  # noqa: E402

TEMP = 0.07
BASE_TEMP = 0.07
NCORES = 8
NVS = 2          # vshards per core
P = 128          # partitions
# |sim| <= (1/TEMP) since features are L2-normalized -> a constant softmax
# shift is numerically safe and removes the per-row reduce_max entirely
CBIAS = 15.0

# populated by kernel() for test harness introspection
LAST_RUN = {}


# ---------------------------------------------------------------- host masks
def _host_masks(labels, labels_queue):
    """Exact replication of the reference's label-only mask evolution."""
    B, L = labels.shape
    Q = labels_queue.shape[0]
    base = int(max(labels.max(), labels_queue.max())) + 1
    pw = base ** np.arange(L - 1, -1, -1)

    anchor_active = np.ones(B, bool)
    queue_active = np.ones(Q, bool)
    order = np.arange(B)

    levels = []
    for l in range(1, L):
        ncols = L - l
        w = (pw * (np.arange(L) < ncols)).astype(np.int64)
        ka = labels.astype(np.int64) @ w
        kq = labels_queue.astype(np.int64) @ w
        maxk = int(max(ka.max(), kq.max())) + 1
        bc = np.bincount(kq[queue_active], minlength=maxk)
        cnt = np.where(anchor_active, bc[ka], 0)
        pres = np.zeros(maxk, bool)
        pres[ka[anchor_active]] = True
        newmatch = queue_active & pres[kq]
        levels.append(dict(
            ka=ka.copy(), kq=kq.copy(),
            queue_active=queue_active.copy(),
            cnt=cnt.copy(),
        ))
        same = (ka[:, None] == ka[None, :]) & anchor_active[:, None] & anchor_active[None, :]
        max_ord = np.max(np.where(same, order[None, :], -1), axis=1)
        kept = anchor_active & (order == max_ord)
        rank = (kept[None, :] & (ka[None, :] < ka[:, None])).sum(1)
        order = np.where(kept, rank, -1)
        anchor_active = kept
        queue_active = queue_active & ~newmatch
    return levels


# ------------------------------------------------------------ device program
def _build_program(D, B, W, n3, n2b):
    NLEV = 3
    f32 = mybir.dt.float32
    bf16 = mybir.dt.bfloat16
    e4 = mybir.dt.float8e4
    DR = mybir.MatmulPerfMode.DoubleRow
    NB = B // P       # anchor chunks
    NK = D // P       # contraction chunks
    NGR = W // 512    # moving groups per vshard

    # possum scan ranges (class-l segment incl the wobble column)
    R3 = (0, min(n3 + 1, W))
    R2 = (n3, min(n2b + 1, W))
    R1 = (n2b, W)
    L3 = R3[1] - R3[0]
    L2 = R2[1] - R2[0]
    L1 = R1[1] - R1[0]

    s2a, s2b = n3 + 1, n2b
    L2s = s2b - s2a

    nc = bacc.Bacc("TRN2", target_bir_lowering=False, debug=False)

    ft_d = nc.dram_tensor("ft", [D, B], e4, kind="ExternalInput").ap()
    fqt_d = nc.dram_tensor("fqt", [D, NVS * W], e4, kind="ExternalInput").ap()
    ka_d = nc.dram_tensor("ka", [NLEV, P, NB], f32, kind="ExternalInput").ap()
    # kq arrays pre-replicated to 128 partitions on host (direct HW-DGE DMA
    # instead of a software-DGE broadcast on the critical path)
    kq3_d = nc.dram_tensor("kq3", [NVS, P, L3], f32, kind="ExternalInput").ap()
    kq2_d = nc.dram_tensor("kq2", [NVS, P, L2], f32, kind="ExternalInput").ap()
    kq1_d = nc.dram_tensor("kq1", [NVS, P, L1], f32, kind="ExternalInput").ap()
    # stats slots: 0:pos3 1:pos2 2:pos1 3:s3 4:xA 5:s2 6:xB 7:s1
    stats_d = nc.dram_tensor(
        "stats", [NVS, P, 8, NB], f32, kind="ExternalOutput").ap()

    with tile.TileContext(nc) as tc, ExitStack() as ctx:
        const_pool = ctx.enter_context(tc.tile_pool(name="const", bufs=1))
        fqt_pool = ctx.enter_context(tc.tile_pool(name="fqt", bufs=2))
        e_pool = ctx.enter_context(tc.tile_pool(name="ee", bufs=2))
        scr_pool = ctx.enter_context(tc.tile_pool(name="scr", bufs=2))
        s3scr_pool = ctx.enter_context(tc.tile_pool(name="s3s", bufs=2))
        st_pool = ctx.enter_context(tc.tile_pool(name="st", bufs=2))
        psum_pool = ctx.enter_context(tc.tile_pool(name="ps", bufs=2, space="PSUM"))

        ft_sb = const_pool.tile([P, NK, B], e4)
        ft_r = ft_d.rearrange("(k p) b -> p k b", p=P)
        # ft DMAs are interleaved with the first vshard's fqt chunks below
        ka_sb = const_pool.tile([P, NLEV, NB], f32)
        nc.gpsimd.dma_start(out=ka_sb, in_=ka_d.rearrange("l p c -> p l c"))
        cbias_sb = const_pool.tile([P, 1], f32)
        nc.vector.memset(cbias_sb, -CBIAS)

        # kq key tiles (direct copies, software DGE queue; issued after the
        # first vshard's fqt DMAs so they don't contend with the first MMs)
        kq3b_all = [const_pool.tile([P, L3], f32, name=f"kq3_{v}")
                    for v in range(NVS)]
        kq2b_all = [const_pool.tile([P, L2], f32, name=f"kq2_{v}")
                    for v in range(NVS)]
        kq1b_all = [const_pool.tile([P, L1], f32, name=f"kq1_{v}")
                    for v in range(NVS)]

        for v in range(NVS):
            fqt_sb = fqt_pool.tile([P, NK, W], e4)
            fqt_r = fqt_d[:, v * W:(v + 1) * W].rearrange("(k p) w -> p k w", p=P)
            for k in range(NK):
                nc.sync.dma_start(out=fqt_sb[:, k, :], in_=fqt_r[:, k, :])
                if v == 0:
                    nc.sync.dma_start(out=ft_sb[:, k, :], in_=ft_r[:, k, :])
            if v == 0:
                for vv in range(NVS):
                    nc.gpsimd.dma_start(out=kq3b_all[vv], in_=kq3_d[vv])
                    nc.gpsimd.dma_start(out=kq2b_all[vv], in_=kq2_d[vv])
                    nc.gpsimd.dma_start(out=kq1b_all[vv], in_=kq1_d[vv])

            kq3b = kq3b_all[v]
            kq2b = kq2b_all[v]
            kq1b = kq1b_all[v]

            stats_v = st_pool.tile([P, 8, NB], f32, tag="stats")

            for c in range(NB):
                ps = psum_pool.tile([P, W], f32)
                for k in range(0, NK, 2):
                    for g in range(NGR):
                        gs = slice(g * 512, (g + 1) * 512)
                        nc.tensor.matmul(
                            ps[:, gs],
                            ft_sb[:, k:k + 2, c * P:(c + 1) * P],
                            fqt_sb[:, k:k + 2, gs],
                            start=(k == 0), stop=(k == NK - 2),
                            perf_mode=DR)

                # E = exp(sim - CBIAS), bf16, one pass over the full width
                E = e_pool.tile([P, W], bf16, tag="E")
                nc.scalar.activation(
                    E, ps, mybir.ActivationFunctionType.Exp,
                    bias=cbias_sb[:, 0:1], scale=1.0)

                # possum per level over its class segment (VectorE)
                scr = scr_pool.tile([P, L3 + L2 + L1], bf16, tag="scr")
                s2scr = s3scr_pool.tile([P, max(L2s, 1)], bf16, tag="s2scr")
                nc.vector.scalar_tensor_tensor(
                    out=scr[:, 0:L3], in0=kq3b,
                    scalar=ka_sb[:, 2, c:c + 1], in1=ps[:, R3[0]:R3[1]],
                    op0=mybir.AluOpType.is_equal, op1=mybir.AluOpType.mult,
                    accum_out=stats_v[:, 0, c:c + 1])
                nc.vector.scalar_tensor_tensor(
                    out=scr[:, L3:L3 + L2], in0=kq2b,
                    scalar=ka_sb[:, 1, c:c + 1], in1=ps[:, R2[0]:R2[1]],
                    op0=mybir.AluOpType.is_equal, op1=mybir.AluOpType.mult,
                    accum_out=stats_v[:, 1, c:c + 1])
                nc.vector.scalar_tensor_tensor(
                    out=scr[:, L3 + L2:L3 + L2 + L1], in0=kq1b,
                    scalar=ka_sb[:, 0, c:c + 1], in1=ps[:, R1[0]:R1[1]],
                    op0=mybir.AluOpType.is_equal, op1=mybir.AluOpType.mult,
                    accum_out=stats_v[:, 2, c:c + 1])

                # denominator segments (disjoint; host recombines with wobble)
                # s3/s1 (small) on VectorE; the big s2 on ScalarE Copy+accum
                nc.vector.tensor_reduce(
                    out=stats_v[:, 3, c:c + 1], in_=E[:, 0:n3],
                    axis=mybir.AxisListType.X, op=mybir.AluOpType.add)
                nc.vector.tensor_reduce(
                    out=stats_v[:, 7, c:c + 1], in_=E[:, n2b + 1:W],
                    axis=mybir.AxisListType.X, op=mybir.AluOpType.add)
                nc.scalar.activation(
                    s2scr[:, 0:L2s], E[:, s2a:s2b],
                    mybir.ActivationFunctionType.Copy,
                    accum_out=stats_v[:, 5, c:c + 1])
                # wobble columns (ScalarE copy)
                nc.scalar.activation(
                    stats_v[:, 4, c:c + 1], E[:, n3:n3 + 1],
                    mybir.ActivationFunctionType.Copy)
                nc.scalar.activation(
                    stats_v[:, 6, c:c + 1], E[:, n2b:n2b + 1],
                    mybir.ActivationFunctionType.Copy)

            nc.sync.dma_start(out=stats_d[v], in_=stats_v)

    nc.compile()
    return nc


# ----------------------------------------------------------------- host prep
def _prepare(features, labels, features_queue, labels_queue):
    B, D = features.shape
    Q = features_queue.shape[0]
    S = NCORES * NVS
    W = Q // S
    NB = B // P
    NLEV = 3

    levels = _host_masks(labels, labels_queue)
    qa2 = levels[1]["queue_active"]
    qa3 = levels[2]["queue_active"]
    life = 1 + qa2.astype(np.int64) + qa3.astype(np.int64)  # 1..3

    order_cols = np.argsort(-life, kind="stable")
    perm = order_cols.reshape(W, S).T  # [S, W]: shard s -> global cols
    life_s = life[perm]
    c3_s = (life_s == 3).sum(1)
    c23_s = (life_s >= 2).sum(1)
    n3 = int(c3_s.min())
    n2b = int(c23_s.min())
    assert int(c3_s.max()) - n3 <= 1 and int(c23_s.max()) - n2b <= 1
    assert 0 < n3 and n3 + 1 < n2b and n2b + 1 < W
    w3 = (c3_s > n3).astype(np.float64)  # [S] wobble col at n3 is class-3
    w2 = (c23_s > n2b).astype(np.float64)

    R3 = (0, n3 + 1)
    R2 = (n3, n2b + 1)
    R1 = (n2b, W)

    # per-level class-masked queue keys (a column can only match at level l
    # if its class is exactly l)
    kq_cls = {}
    for li, cls in ((0, 1), (1, 2), (2, 3)):
        k = levels[li]["kq"].astype(np.float32)
        kq_cls[li] = np.where(life == cls, k, np.float32(-1.0))[perm]  # [S, W]

    ka_r = np.empty((NLEV, P, NB), np.float32)
    for li in range(NLEV):
        ka_r[li] = levels[li]["ka"].astype(np.float32).reshape(NB, P).T

    e4 = ml_dtypes.float8_e4m3
    ft8 = np.ascontiguousarray((features / TEMP).T).astype(e4)  # [D, B]
    fqT = np.ascontiguousarray(features_queue.T)                # [D, Q]

    in_maps = []
    for cidx in range(NCORES):
        sh = range(cidx * NVS, (cidx + 1) * NVS)
        cols = np.concatenate([perm[s] for s in sh])
        fq8 = np.ascontiguousarray(fqT[:, cols]).astype(e4)
        def _rep(a):  # [NVS, L] -> [NVS, P, L] replicated over partitions
            return np.ascontiguousarray(
                np.broadcast_to(a[:, None, :], (NVS, P, a.shape[1])))
        m = {
            "ft": ft8, "fqt": fq8, "ka": ka_r,
            "kq3": _rep(np.stack([kq_cls[2][s, R3[0]:R3[1]] for s in sh])),
            "kq2": _rep(np.stack([kq_cls[1][s, R2[0]:R2[1]] for s in sh])),
            "kq1": _rep(np.stack([kq_cls[0][s, R1[0]:R1[1]] for s in sh])),
        }
        in_maps.append(m)

    return dict(in_maps=in_maps, levels=levels, perm=perm,
                n3=n3, n2b=n2b, w3=w3, w2=w2,
                B=B, D=D, Q=Q, S=S, W=W, NB=NB, NLEV=NLEV)


# -------------------------------------------------------------------- kernel
def kernel(features, labels, features_queue, labels_queue):
    t0 = time.time()
    features = np.asarray(features, dtype=np.float32)
    features_queue = np.asarray(features_queue, dtype=np.float32)
    labels = np.asarray(labels)
    labels_queue = np.asarray(labels_queue)

    prep = _prepare(features, labels, features_queue, labels_queue)
    in_maps = prep["in_maps"]
    levels = prep["levels"]
    B, D = prep["B"], prep["D"]
    S, W = prep["S"], prep["W"]
    NB, NLEV = prep["NB"], prep["NLEV"]
    n3, n2b = prep["n3"], prep["n2b"]
    w3, w2 = prep["w3"], prep["w2"]
    t_prep = time.time() - t0

    t0 = time.time()
    nc = _build_program(D, B, W, n3, n2b)
    t_build = time.time() - t0

    t0 = time.time()
    br = run_bass_kernel_spmd(nc, in_maps, core_ids=list(range(NCORES)))
    t_run = time.time() - t0

    LAST_RUN.clear()
    LAST_RUN.update(
        exec_time_ns=br.exec_time_ns,
        mean_exec_time_ns=getattr(br, "mean_exec_time_ns", None),
        t_prep=t_prep, t_build=t_build, t_run=t_run,
        profile_json=br.profile_json,
        instructions_and_trace=br.instructions_and_trace,
        n3=n3, n2b=n2b)

    # ---- host merge (float64)
    t0 = time.time()
    pos = np.empty((S, NLEV, B), np.float64)
    den = np.empty((S, NLEV, B), np.float64)
    for cidx in range(NCORES):
        st = br.results[cidx]["stats"].astype(np.float64)  # [NVS, P, 8, NB]
        for v in range(NVS):
            s = cidx * NVS + v
            sv = st[v]  # [P, 8, NB]
            # slots: 0:pos3 1:pos2 2:pos1 3:s3 4:xA 5:s2 6:xB 7:s1
            for li, slot in ((2, 0), (1, 1), (0, 2)):
                pos[s, li] = sv[:, slot, :].T.reshape(-1)
            s3 = sv[:, 3, :].T.reshape(-1)
            xA = sv[:, 4, :].T.reshape(-1)
            s2 = sv[:, 5, :].T.reshape(-1)
            xB = sv[:, 6, :].T.reshape(-1)
            s1 = sv[:, 7, :].T.reshape(-1)
            den[s, 2] = s3 + w3[s] * xA
            den[s, 1] = s3 + xA + s2 + w2[s] * xB
            den[s, 0] = s3 + xA + s2 + xB + s1

    cum = 0.0
    max_lower = -np.inf
    for li in range(NLEV):
        l = li + 1
        cnt = levels[li]["cnt"].astype(np.float64)
        dtot = den[:, li, :].sum(axis=0)
        ptot = pos[:, li, :].sum(axis=0)
        with np.errstate(divide="ignore", invalid="ignore"):
            mean = (ptot - cnt * (CBIAS + np.log(dtot))) / (cnt + 1e-12)
        mean = np.where(cnt > 0, mean, 0.0)
        loss_i = -(TEMP / BASE_TEMP) * mean
        num = float((cnt > 0).sum())
        layer_loss = float(loss_i.sum() / (num + 1e-12))
        layer_loss = max(max_lower, layer_loss)
        cum = cum + (2.0 ** (1.0 / l)) * layer_loss
        max_lower = max(max_lower, layer_loss)

    LAST_RUN["t_merge"] = time.time() - t0
    return np.float32(cum)


# revision 51
# speedup vs baseline: 1.4836x; 1.0673x over previous
"""HMLC loss kernel for 8 Trainium2 NeuronCores (Bass/Tile).

Strategy (queue-sharded data parallelism, fp8 matmul, class-segment stats):
  * All mask/dedup/queue-evolution logic in the reference depends ONLY on the
    integer labels -> computed exactly on host (numpy).
  * The queue (32768 cols) is split into 16 shards (8 cores x 2 vshards).
    Within each shard, columns are ordered by "lifetime class" (the last
    level at which the column is still active, 3..1), assigned round-robin
    over the class-sorted global order so per-class counts differ by at most
    1 across shards ("wobble" of one column at each class boundary).
  * Key fact used for the possum scans: a queue column can be matched at
    level l by SOME anchor iff it dies after level l (class == l); and
    match(i,j) at level l != 0 implies column j is matched at level l.
    Hence pos_l = sum_j (kq_l==ka_i)*sim over ONLY the class-l segment.
  * Device per (vshard, anchor-chunk): PE computes sim = (f/TEMP) @ fq_shard.T
    into PSUM [128, 2048] via fp8e4 DoubleRow matmuls (fp8 noise averages
    out; final loss rel err ~1e-5); then
        E      = exp(sim - CBIAS)            (ScalarE, bf16 out, 1 pass)
        pos_l  = stt (kq_l==ka)*sim, class-l segment   (VectorE, accum)
        s3/s2/s1 = segment sums of E          (VectorE tensor_reduce)
        xA/xB  = E at the two wobble columns  (ScalarE copy)
  * Host combines segments + wobble flags into exact per-shard denominators,
    merges shards in float64, and runs the scalar hmce chain.
"""

import os
import sys
import time
from contextlib import ExitStack

if "/opt/trn_rl_repo" not in sys.path:
    sys.path.insert(0, "/opt/trn_rl_repo")

import numpy as np
import ml_dtypes

import concourse.bass as bass  # noqa: E402
import concourse.bacc as bacc  # noqa: E402
import concourse.tile as tile  # noqa: E402
from concourse import mybir  # noqa: E402
from concourse.bass_utils import run_bass_kernel_spmd  # noqa: E402

TEMP = 0.07
BASE_TEMP = 0.07
NCORES = 8
NVS = 2          # vshards per core
P = 128          # partitions
# |sim| <= (1/TEMP) since features are L2-normalized -> a constant softmax
# shift is numerically safe and removes the per-row reduce_max entirely
CBIAS = 15.0

# populated by kernel() for test harness introspection
LAST_RUN = {}


# ---------------------------------------------------------------- host masks
def _host_masks(labels, labels_queue):
    """Exact replication of the reference's label-only mask evolution."""
    B, L = labels.shape
    Q = labels_queue.shape[0]
    base = int(max(labels.max(), labels_queue.max())) + 1
    pw = base ** np.arange(L - 1, -1, -1)

    anchor_active = np.ones(B, bool)
    queue_active = np.ones(Q, bool)
    order = np.arange(B)

    levels = []
    for l in range(1, L):
        ncols = L - l
        w = (pw * (np.arange(L) < ncols)).astype(np.int64)
        ka = labels.astype(np.int64) @ w
        kq = labels_queue.astype(np.int64) @ w
        maxk = int(max(ka.max(), kq.max())) + 1
        bc = np.bincount(kq[queue_active], minlength=maxk)
        cnt = np.where(anchor_active, bc[ka], 0)
        pres = np.zeros(maxk, bool)
        pres[ka[anchor_active]] = True
        newmatch = queue_active & pres[kq]
        levels.append(dict(
            ka=ka.copy(), kq=kq.copy(),
            queue_active=queue_active.copy(),
            cnt=cnt.copy(),
        ))
        same = (ka[:, None] == ka[None, :]) & anchor_active[:, None] & anchor_active[None, :]
        max_ord = np.max(np.where(same, order[None, :], -1), axis=1)
        kept = anchor_active & (order == max_ord)
        rank = (kept[None, :] & (ka[None, :] < ka[:, None])).sum(1)
        order = np.where(kept, rank, -1)
        anchor_active = kept
        queue_active = queue_active & ~newmatch
    return levels


# ------------------------------------------------------------ device program
def _build_program(D, B, W, n3, n2b):
    NLEV = 3
    f32 = mybir.dt.float32
    bf16 = mybir.dt.bfloat16
    e4 = mybir.dt.float8e4
    DR = mybir.MatmulPerfMode.DoubleRow
    NB = B // P       # anchor chunks
    NK = D // P       # contraction chunks
    NGR = W // 512    # moving groups per vshard

    # possum scan ranges (class-l segment incl the wobble column)
    R3 = (0, min(n3 + 1, W))
    R2 = (n3, min(n2b + 1, W))
    R1 = (n2b, W)
    L3 = R3[1] - R3[0]
    L2 = R2[1] - R2[0]
    L1 = R1[1] - R1[0]

    s2a, s2b = n3 + 1, n2b
    L2s = s2b - s2a

    nc = bacc.Bacc("TRN2", target_bir_lowering=False, debug=False)

    ft_d = nc.dram_tensor("ft", [D, B], e4, kind="ExternalInput").ap()
    fqt_d = nc.dram_tensor("fqt", [D, NVS * W], e4, kind="ExternalInput").ap()
    ka_d = nc.dram_tensor("ka", [NLEV, P, NB], f32, kind="ExternalInput").ap()
    # kq arrays pre-replicated to 128 partitions on host (direct HW-DGE DMA
    # instead of a software-DGE broadcast on the critical path)
    kq3_d = nc.dram_tensor("kq3", [NVS, P, L3], f32, kind="ExternalInput").ap()
    kq2_d = nc.dram_tensor("kq2", [NVS, P, L2], f32, kind="ExternalInput").ap()
    kq1_d = nc.dram_tensor("kq1", [NVS, P, L1], f32, kind="ExternalInput").ap()
    # stats slots: 0:pos3 1:pos2 2:pos1 3:s3 4:xA 5:s2 6:xB 7:s1
    stats_d = nc.dram_tensor(
        "stats", [NVS, P, 8, NB], f32, kind="ExternalOutput").ap()

    with tile.TileContext(nc) as tc, ExitStack() as ctx:
        const_pool = ctx.enter_context(tc.tile_pool(name="const", bufs=1))
        fqt_pool = ctx.enter_context(tc.tile_pool(name="fqt", bufs=2))
        e_pool = ctx.enter_context(tc.tile_pool(name="ee", bufs=2))
        scr_pool = ctx.enter_context(tc.tile_pool(name="scr", bufs=2))
        s3scr_pool = ctx.enter_context(tc.tile_pool(name="s3s", bufs=2))
        st_pool = ctx.enter_context(tc.tile_pool(name="st", bufs=2))
        psum_pool = ctx.enter_context(tc.tile_pool(name="ps", bufs=2, space="PSUM"))

        ft_sb = const_pool.tile([P, NK, B], e4)
        ft_r = ft_d.rearrange("(k p) b -> p k b", p=P)
        # ft DMAs are interleaved with the first vshard's fqt chunks below
        ka_sb = const_pool.tile([P, NLEV, NB], f32)
        nc.gpsimd.dma_start(out=ka_sb, in_=ka_d.rearrange("l p c -> p l c"))
        cbias_sb = const_pool.tile([P, 1], f32)
        nc.vector.memset(cbias_sb, -CBIAS)

        # kq key tiles (direct copies, software DGE queue; issued after the
        # first vshard's fqt DMAs so they don't contend with the first MMs)
        kq3b_all = [const_pool.tile([P, L3], f32, name=f"kq3_{v}")
                    for v in range(NVS)]
        kq2b_all = [const_pool.tile([P, L2], f32, name=f"kq2_{v}")
                    for v in range(NVS)]
        kq1b_all = [const_pool.tile([P, L1], f32, name=f"kq1_{v}")
                    for v in range(NVS)]

        for v in range(NVS):
            fqt_sb = fqt_pool.tile([P, NK, W], e4)
            fqt_r = fqt_d[:, v * W:(v + 1) * W].rearrange("(k p) w -> p k w", p=P)
            for k in range(NK):
                nc.sync.dma_start(out=fqt_sb[:, k, :], in_=fqt_r[:, k, :])
                if v == 0:
                    # parallel HW-DGE queue so ft and fqt stream concurrently
                    nc.scalar.dma_start(out=ft_sb[:, k, :], in_=ft_r[:, k, :])
            if v == 0:
                for vv in range(NVS):
                    nc.gpsimd.dma_start(out=kq3b_all[vv], in_=kq3_d[vv])
                    nc.gpsimd.dma_start(out=kq2b_all[vv], in_=kq2_d[vv])
                    nc.gpsimd.dma_start(out=kq1b_all[vv], in_=kq1_d[vv])

            kq3b = kq3b_all[v]
            kq2b = kq2b_all[v]
            kq1b = kq1b_all[v]

            stats_v = st_pool.tile([P, 8, NB], f32, tag="stats")

            for c in range(NB):
                ps = psum_pool.tile([P, W], f32)
                for k in range(0, NK, 2):
                    for g in range(NGR):
                        gs = slice(g * 512, (g + 1) * 512)
                        nc.tensor.matmul(
                            ps[:, gs],
                            ft_sb[:, k:k + 2, c * P:(c + 1) * P],
                            fqt_sb[:, k:k + 2, gs],
                            start=(k == 0), stop=(k == NK - 2),
                            perf_mode=DR)

                # E = exp(sim - CBIAS), bf16, one pass over the full width
                E = e_pool.tile([P, W], bf16, tag="E")
                nc.scalar.activation(
                    E, ps, mybir.ActivationFunctionType.Exp,
                    bias=cbias_sb[:, 0:1], scale=1.0)

                # possum per level over its class segment (VectorE)
                scr = scr_pool.tile([P, L3 + L2 + L1], bf16, tag="scr")
                s2scr = s3scr_pool.tile([P, max(L2s, 1)], bf16, tag="s2scr")
                nc.vector.scalar_tensor_tensor(
                    out=scr[:, 0:L3], in0=kq3b,
                    scalar=ka_sb[:, 2, c:c + 1], in1=ps[:, R3[0]:R3[1]],
                    op0=mybir.AluOpType.is_equal, op1=mybir.AluOpType.mult,
                    accum_out=stats_v[:, 0, c:c + 1])
                nc.vector.scalar_tensor_tensor(
                    out=scr[:, L3:L3 + L2], in0=kq2b,
                    scalar=ka_sb[:, 1, c:c + 1], in1=ps[:, R2[0]:R2[1]],
                    op0=mybir.AluOpType.is_equal, op1=mybir.AluOpType.mult,
                    accum_out=stats_v[:, 1, c:c + 1])
                nc.vector.scalar_tensor_tensor(
                    out=scr[:, L3 + L2:L3 + L2 + L1], in0=kq1b,
                    scalar=ka_sb[:, 0, c:c + 1], in1=ps[:, R1[0]:R1[1]],
                    op0=mybir.AluOpType.is_equal, op1=mybir.AluOpType.mult,
                    accum_out=stats_v[:, 2, c:c + 1])

                # denominator segments (disjoint; host recombines with wobble)
                # s3/s1 (small) on VectorE; the big s2 on ScalarE Copy+accum
                nc.vector.tensor_reduce(
                    out=stats_v[:, 3, c:c + 1], in_=E[:, 0:n3],
                    axis=mybir.AxisListType.X, op=mybir.AluOpType.add)
                nc.vector.tensor_reduce(
                    out=stats_v[:, 7, c:c + 1], in_=E[:, n2b + 1:W],
                    axis=mybir.AxisListType.X, op=mybir.AluOpType.add)
                nc.scalar.activation(
                    s2scr[:, 0:L2s], E[:, s2a:s2b],
                    mybir.ActivationFunctionType.Copy,
                    accum_out=stats_v[:, 5, c:c + 1])
                # wobble columns (VectorE width-1 reduces = cheap copies)
                nc.vector.tensor_reduce(
                    out=stats_v[:, 4, c:c + 1], in_=E[:, n3:n3 + 1],
                    axis=mybir.AxisListType.X, op=mybir.AluOpType.add)
                nc.vector.tensor_reduce(
                    out=stats_v[:, 6, c:c + 1], in_=E[:, n2b:n2b + 1],
                    axis=mybir.AxisListType.X, op=mybir.AluOpType.add)

            # gpsimd queue: keeps the sync queue free for the next vshard's
            # fqt loads (in-order queue would stall behind this stats wait)
            nc.gpsimd.dma_start(out=stats_d[v], in_=stats_v)

    nc.compile()
    return nc


# ----------------------------------------------------------------- host prep
def _prepare(features, labels, features_queue, labels_queue):
    B, D = features.shape
    Q = features_queue.shape[0]
    S = NCORES * NVS
    W = Q // S
    NB = B // P
    NLEV = 3

    levels = _host_masks(labels, labels_queue)
    qa2 = levels[1]["queue_active"]
    qa3 = levels[2]["queue_active"]
    life = 1 + qa2.astype(np.int64) + qa3.astype(np.int64)  # 1..3

    order_cols = np.argsort(-life, kind="stable")
    perm = order_cols.reshape(W, S).T  # [S, W]: shard s -> global cols
    life_s = life[perm]
    c3_s = (life_s == 3).sum(1)
    c23_s = (life_s >= 2).sum(1)
    n3 = int(c3_s.min())
    n2b = int(c23_s.min())
    assert int(c3_s.max()) - n3 <= 1 and int(c23_s.max()) - n2b <= 1
    assert 0 < n3 and n3 + 1 < n2b and n2b + 1 < W
    w3 = (c3_s > n3).astype(np.float64)  # [S] wobble col at n3 is class-3
    w2 = (c23_s > n2b).astype(np.float64)

    R3 = (0, n3 + 1)
    R2 = (n3, n2b + 1)
    R1 = (n2b, W)

    # per-level class-masked queue keys (a column can only match at level l
    # if its class is exactly l)
    kq_cls = {}
    for li, cls in ((0, 1), (1, 2), (2, 3)):
        k = levels[li]["kq"].astype(np.float32)
        kq_cls[li] = np.where(life == cls, k, np.float32(-1.0))[perm]  # [S, W]

    ka_r = np.empty((NLEV, P, NB), np.float32)
    for li in range(NLEV):
        ka_r[li] = levels[li]["ka"].astype(np.float32).reshape(NB, P).T

    e4 = ml_dtypes.float8_e4m3
    ft8 = np.ascontiguousarray((features / TEMP).T).astype(e4)  # [D, B]
    fqT = np.ascontiguousarray(features_queue.T)                # [D, Q]

    in_maps = []
    for cidx in range(NCORES):
        sh = range(cidx * NVS, (cidx + 1) * NVS)
        cols = np.concatenate([perm[s] for s in sh])
        fq8 = np.ascontiguousarray(fqT[:, cols]).astype(e4)
        def _rep(a):  # [NVS, L] -> [NVS, P, L] replicated over partitions
            return np.ascontiguousarray(
                np.broadcast_to(a[:, None, :], (NVS, P, a.shape[1])))
        m = {
            "ft": ft8, "fqt": fq8, "ka": ka_r,
            "kq3": _rep(np.stack([kq_cls[2][s, R3[0]:R3[1]] for s in sh])),
            "kq2": _rep(np.stack([kq_cls[1][s, R2[0]:R2[1]] for s in sh])),
            "kq1": _rep(np.stack([kq_cls[0][s, R1[0]:R1[1]] for s in sh])),
        }
        in_maps.append(m)

    return dict(in_maps=in_maps, levels=levels, perm=perm,
                n3=n3, n2b=n2b, w3=w3, w2=w2,
                B=B, D=D, Q=Q, S=S, W=W, NB=NB, NLEV=NLEV)


# -------------------------------------------------------------------- kernel
def kernel(features, labels, features_queue, labels_queue):
    t0 = time.time()
    features = np.asarray(features, dtype=np.float32)
    features_queue = np.asarray(features_queue, dtype=np.float32)
    labels = np.asarray(labels)
    labels_queue = np.asarray(labels_queue)

    prep = _prepare(features, labels, features_queue, labels_queue)
    in_maps = prep["in_maps"]
    levels = prep["levels"]
    B, D = prep["B"], prep["D"]
    S, W = prep["S"], prep["W"]
    NB, NLEV = prep["NB"], prep["NLEV"]
    n3, n2b = prep["n3"], prep["n2b"]
    w3, w2 = prep["w3"], prep["w2"]
    t_prep = time.time() - t0

    t0 = time.time()
    nc = _build_program(D, B, W, n3, n2b)
    t_build = time.time() - t0

    t0 = time.time()
    br = run_bass_kernel_spmd(nc, in_maps, core_ids=list(range(NCORES)))
    t_run = time.time() - t0

    LAST_RUN.clear()
    LAST_RUN.update(
        exec_time_ns=br.exec_time_ns,
        mean_exec_time_ns=getattr(br, "mean_exec_time_ns", None),
        t_prep=t_prep, t_build=t_build, t_run=t_run,
        profile_json=br.profile_json,
        instructions_and_trace=br.instructions_and_trace,
        n3=n3, n2b=n2b)

    # ---- host merge (float64)
    t0 = time.time()
    pos = np.empty((S, NLEV, B), np.float64)
    den = np.empty((S, NLEV, B), np.float64)
    for cidx in range(NCORES):
        st = br.results[cidx]["stats"].astype(np.float64)  # [NVS, P, 8, NB]
        for v in range(NVS):
            s = cidx * NVS + v
            sv = st[v]  # [P, 8, NB]
            # slots: 0:pos3 1:pos2 2:pos1 3:s3 4:xA 5:s2 6:xB 7:s1
            for li, slot in ((2, 0), (1, 1), (0, 2)):
                pos[s, li] = sv[:, slot, :].T.reshape(-1)
            s3 = sv[:, 3, :].T.reshape(-1)
            xA = sv[:, 4, :].T.reshape(-1)
            s2 = sv[:, 5, :].T.reshape(-1)
            xB = sv[:, 6, :].T.reshape(-1)
            s1 = sv[:, 7, :].T.reshape(-1)
            den[s, 2] = s3 + w3[s] * xA
            den[s, 1] = s3 + xA + s2 + w2[s] * xB
            den[s, 0] = s3 + xA + s2 + xB + s1

    cum = 0.0
    max_lower = -np.inf
    for li in range(NLEV):
        l = li + 1
        cnt = levels[li]["cnt"].astype(np.float64)
        dtot = den[:, li, :].sum(axis=0)
        ptot = pos[:, li, :].sum(axis=0)
        with np.errstate(divide="ignore", invalid="ignore"):
            mean = (ptot - cnt * (CBIAS + np.log(dtot))) / (cnt + 1e-12)
        mean = np.where(cnt > 0, mean, 0.0)
        loss_i = -(TEMP / BASE_TEMP) * mean
        num = float((cnt > 0).sum())
        layer_loss = float(loss_i.sum() / (num + 1e-12))
        layer_loss = max(max_lower, layer_loss)
        cum = cum + (2.0 ** (1.0 / l)) * layer_loss
        max_lower = max(max_lower, layer_loss)

    LAST_RUN["t_merge"] = time.time() - t0
    return np.float32(cum)
